# revision 26
# baseline (speedup 1.0000x reference)
"""FFTConv2d kernel for trn2, 8 NeuronCores.

Math: reference einsum 'bchw,oihw->bohw' factorizes:
  Y[b,o] = conv_full(sum_c x[b,c], sum_i w[o,i])[1:-1,1:-1] + bias[o]
i.e. a single-channel 3x3 "same" convolution (flipped kernel) per (b,o).

v5: fp16 end-to-end (PSUM fp32), uneven row-slices [8,40,40,40], and an
HBM round-trip to build the conv operand (only 6 DMAs per slice, no
SBUF->SBUF traffic). Per core (2 batches), per slice:
  1. DMA x slice in as fp16, partitions=(b,c), SH+2 row-slots with memset
     zero-rows at the image edges.
  2. Channel-sum matmul (ones lhsT), 4x col-tiled via tile_position
     (0,32g): phase g covers SH/4 output rows; psum partitions 32g+b.
  3. Copy PSUM -> phase-split staging fp16 (row stride 130, pad cols
     memset once per buffer), one [128,512] copy per 4-slot chunk.
  4. S-out: 2 DMAs (per b) write the padded channel-sum image to an HBM
     scratch slab, merging the 4 phases via overlapping halo writes.
  5. P9-in: 2 DMAs (per b) read back P9 [19, SH*130]: flat HBM src AP
     [[130,3],[1,3],[1,PWIN]] encodes both 3x3 shifts; P9 partition
     9b + 3(2-u) + (2-v). Partition 18 holds ones (bias row).
  6. Conv: per <=3-row chunk one fp16 matmul wb[19,128].T @ P9 window ->
     PSUM [128,3,130].
  7. Copy PSUM -> yt fp16 (drop pad cols), DMA yt -> HBM; host upcasts.
Conv trails channel-sum by 2 slices (DEPTH=2).
"""

import os
import sys
from functools import lru_cache

import numpy as np

for _p in ("/opt/trn_rl_repo", "/root/.axon_site/_ro/trn_rl_repo"):
    if os.path.isdir(_p) and _p not in sys.path:
        sys.path.insert(0, _p)

import ml_dtypes

B, CIN, COUT, H, W = 16, 64, 64, 128, 128
N_CORES = 8
BPC = B // N_CORES  # batches per core = 2
WROW = W + 2  # padded row stride = 130
NPH = 4  # col-tile phases per slice
NPART = BPC * CIN  # 128 input partitions (b, c)
NOUT = BPC * COUT  # 128 output partitions (b, o)
KCONV = BPC * 9 + 1  # 19 conv contraction rows

SLICE_SH = [8, 48, 48, 24]  # output rows per slice
NS = len(SLICE_SH)
SH_MAX = max(SLICE_SH)
PSLOT_MAX = SH_MAX // NPH + 2  # 16
PHLEN = PSLOT_MAX * WROW + 2  # staging cols = 1562
ST2 = (SH_MAX + 2) * WROW + 2  # HBM scratch cols per (b, slice) = 5462
PWIN_MAX = SH_MAX * WROW  # 5200

_SLICE_R0 = np.cumsum([0] + SLICE_SH).tolist()  # row starts
_SLICE_ROWS = []  # clipped input rows
_SLICE_OFF = []
_off = 0
for _s in range(NS):
    _h0 = max(0, _SLICE_R0[_s] - 1)
    _he = min(H, _SLICE_R0[_s + 1] + 1)
    _SLICE_ROWS.append((_h0, _he))
    _SLICE_OFF.append(_off)
    _off += (_he - _h0) * W
XPACK_LEN = _off


@lru_cache(maxsize=1)
def _build():
    import concourse.bacc as bacc
    import concourse.mybir as mybir
    import concourse.tile as tile
    from concourse.ap import AP

    f32 = mybir.dt.float32
    f16 = mybir.dt.float16

    nc = bacc.Bacc("TRN2", target_bir_lowering=False, debug=False, num_devices=N_CORES)

    xp = nc.dram_tensor("xpack", [NPART, XPACK_LEN], f16, kind="ExternalInput")
    ones_cs = nc.dram_tensor("ones_cs", [NPART, BPC], f16, kind="ExternalInput")
    wb = nc.dram_tensor("wb", [128, NOUT], f16, kind="ExternalInput")
    ones_p = nc.dram_tensor("ones_p", [1, PWIN_MAX], f16, kind="ExternalInput")
    y = nc.dram_tensor("y", [NOUT, H * W], f16, kind="ExternalOutput")
    # HBM scratch: padded channel-sum image per (b, slice)
    ssc = nc.dram_tensor("ssc", [1, BPC * NS * ST2], f16, kind="Internal")

    with tile.TileContext(nc) as tc:
        with (
            tc.tile_pool(name="xin", bufs=4) as xin_pool,
            tc.tile_pool(name="sp", bufs=1) as sp_pool,
            tc.tile_pool(name="pbuf", bufs=1) as p_pool,
            tc.tile_pool(name="yout", bufs=2) as y_pool,
            tc.tile_pool(name="consts", bufs=1) as c_pool,
            tc.tile_pool(name="cs_ps", bufs=1, space="PSUM") as cs_psum,
            tc.tile_pool(name="cv_ps", bufs=4, space="PSUM") as cv_psum,
        ):
            def emit_in(s):
                h0, he = _SLICE_ROWS[s]
                ncols = (he - h0) * W
                nslot = SLICE_SH[s] + 2
                xin = xin_pool.tile([NPART, nslot * W], f16, tag="xin")
                o = _SLICE_OFF[s]
                d0 = (h0 - (_SLICE_R0[s] - 1)) * W  # W for s=0 else 0
                if s == 0:
                    nc.vector.memset(xin[:, 0:W], 0.0)
                if s == NS - 1:
                    nc.vector.memset(xin[:, d0 + ncols :], 0.0)
                nc.scalar.dma_start(
                    out=xin[:, d0 : d0 + ncols], in_=xp.ap()[:, o : o + ncols]
                )
                return xin

            # slice-0 input first for the fastest pipeline start
            xins = {}
            xins[0] = emit_in(0)
            ones_t = c_pool.tile([NPART, BPC], f16, tag="ones_cs")
            nc.scalar.dma_start(out=ones_t[:, :], in_=ones_cs.ap()[:, :])
            wb_t = c_pool.tile([128, NOUT], f16, tag="wb")
            nc.scalar.dma_start(out=wb_t[:, :], in_=wb.ap()[:, :])
            for s in range(1, NS):
                xins[s] = emit_in(s)

            NBUF = 2
            NBUF9 = 3
            spbufs = []
            p9bufs = []
            for pi in range(NBUF):
                sp = sp_pool.tile([NPART, PHLEN], f16, tag=f"SP{pi}")
                spt = sp.tensor
                nc.vector.memset(sp[:, 0:1], 0.0)
                nc.vector.memset(
                    AP(tensor=spt, offset=WROW - 1,
                       ap=[[PHLEN, NPART], [WROW, PSLOT_MAX], [1, 2]]),
                    0.0,
                )
                nc.vector.memset(sp[:, PHLEN - 1 : PHLEN], 0.0)
                spbufs.append(sp)
            for pi in range(NBUF9):
                p9 = p_pool.tile([64 + KCONV, PWIN_MAX], f16, tag=f"P9{pi}")
                nc.sync.dma_start(
                    out=p9[KCONV - 1 : KCONV, :], in_=ones_p.ap()[0:1, :]
                )
                nc.sync.dma_start(
                    out=p9[64 + KCONV - 1 : 64 + KCONV, :],
                    in_=ones_p.ap()[0:1, :],
                )
                p9bufs.append(p9)

            def cs_thunks(s, xin):
                sh = SLICE_SH[s]
                prows = sh // NPH
                pslot = prows + 2
                nchunk = (pslot + 3) // 4
                sp = spbufs[s % NBUF]
                spt = sp.tensor
                p9 = p9bufs[s % NBUF9]

                def chunk(cb):
                    sl0 = 4 * cb
                    nrows = min(4, pslot - sl0)
                    n = nrows * W
                    ps = cs_psum.tile([NPART, 4, W], f32, tag=f"cs{cb}",
                                      name=f"cs{s}_{cb}")
                    for g in range(NPH):
                        c0 = (g * prows + sl0) * W
                        nc.tensor.matmul(
                            ps[32 * g : 32 * g + BPC, :nrows, :],
                            ones_t[:, :],
                            xin[:, c0 : c0 + n],
                            start=True,
                            stop=True,
                            tile_position=(0, 32 * g),
                        )
                    dst = AP(
                        tensor=spt,
                        offset=sl0 * WROW + 1,
                        ap=[[PHLEN, NPART], [WROW, nrows], [1, W]],
                    )
                    if cb == 1:
                        nc.vector.tensor_copy(dst, ps[:, :nrows, :])
                    else:
                        nc.scalar.copy(dst, ps[:, :nrows, :])

                def tail():
                    # S-out then P9-in via HBM scratch
                    pwin = sh * WROW
                    for b in range(BPC):
                        base = (b * NS + s) * ST2
                        nc.gpsimd.dma_start(
                            out=AP(
                                tensor=ssc,
                                offset=base,
                                ap=[[prows * WROW, NPH], [1, pslot * WROW]],
                            ),
                            in_=AP(
                                tensor=spt,
                                offset=b * PHLEN,
                                ap=[[32 * PHLEN, NPH], [1, pslot * WROW]],
                            ),
                        )
                    for G in range(2):
                        for b in range(BPC):
                            base = (b * NS + s) * ST2
                            nc.sync.dma_start(
                                out=p9[64 * G + 9 * b : 64 * G + 9 * b + 9,
                                       0:pwin],
                                in_=AP(
                                    tensor=ssc,
                                    offset=base,
                                    ap=[[WROW, 3], [1, 3], [1, pwin]],
                                ),
                            )

                return [lambda cb=cb: chunk(cb) for cb in range(nchunk)] + [tail], p9

            def cv_thunks(s, p9):
                sh = SLICE_SH[s]
                r0 = _SLICE_R0[s]
                yt = y_pool.tile([NOUT, sh, W], f16, tag="yout",
                                 name=f"yt{s}")
                nchunk = (sh + 2) // 3
                half = (nchunk + 1) // 2
                hrow = half * 3

                def chunk(c):
                    rr0 = c * 3
                    nrr = min(3, sh - rr0)
                    G = 64 * (c % 2)
                    ps = cv_psum.tile([NOUT, 3, WROW], f32, tag="cv",
                                      name=f"cv{s}_{c}")
                    nc.tensor.matmul(
                        ps[:, :nrr, :],
                        wb_t[G : G + KCONV, :],
                        p9[G : G + KCONV, rr0 * WROW : (rr0 + nrr) * WROW],
                        start=True,
                        stop=True,
                        tile_position=(G, 0),
                    )
                    if c % 2 == 0:
                        nc.vector.tensor_copy(
                            yt[:, rr0 : rr0 + nrr, :], ps[:, :nrr, 0:W]
                        )
                    else:
                        nc.scalar.copy(yt[:, rr0 : rr0 + nrr, :],
                                       ps[:, :nrr, 0:W])
                    if sh > 8 and c == half - 1:
                        nc.gpsimd.dma_start(
                            out=y.ap()[:, r0 * W : (r0 + hrow) * W],
                            in_=yt[:, :hrow, :],
                        )
                    if c == nchunk - 1:
                        a = hrow if sh > 8 else 0
                        nc.gpsimd.dma_start(
                            out=y.ap()[:, (r0 + a) * W : (r0 + sh) * W],
                            in_=yt[:, a:, :],
                        )

                return [lambda c=c: chunk(c) for c in range(nchunk)]

            DEPTH = 2
            p9s = {}
            for s in range(NS + DEPTH):
                if s < NS:
                    A, p9s[s] = cs_thunks(s, xins[s])
                    for t in A:
                        t()
                if s >= DEPTH:
                    for t in cv_thunks(s - DEPTH, p9s[s - DEPTH]):
                        t()

    nc.compile()
    return nc


def _host_prep(x, weight, bias):
    fh = np.float16
    wsum = weight.sum(axis=1)  # [COUT, 3, 3] fp32
    wb = np.zeros((128, NOUT), np.float32)
    for b in range(BPC):
        for u in range(3):
            for v in range(3):
                wb[9 * b + 3 * (2 - u) + (2 - v),
                   b * COUT : (b + 1) * COUT] = wsum[:, u, v]
    wb[KCONV - 1, :] = np.tile(bias, BPC)
    wb[64 : 64 + KCONV, :] = wb[0:KCONV, :]
    wb = wb.astype(fh)
    ones_cs = np.zeros((NPART, BPC), np.float32)
    for b in range(BPC):
        ones_cs[b * CIN : (b + 1) * CIN, b] = 1.0
    ones_cs = ones_cs.astype(fh)
    ones_p = np.ones((1, PWIN_MAX), dtype=fh)

    in_maps = []
    for r in range(N_CORES):
        xs = np.ascontiguousarray(
            x[r * BPC : (r + 1) * BPC].reshape(NPART, H, W)
        ).astype(fh)
        xpack = np.empty((NPART, XPACK_LEN), dtype=fh)
        for s in range(NS):
            h0, he = _SLICE_ROWS[s]
            n = (he - h0) * W
            o = _SLICE_OFF[s]
            xpack[:, o : o + n] = xs[:, h0:he].reshape(NPART, n)
        in_maps.append(
            {
                "xpack": xpack,
                "ones_cs": ones_cs,
                "wb": wb,
                "ones_p": ones_p,
            }
        )
    return in_maps


def kernel(x, weight, bias):
    from concourse.bass_utils import run_bass_kernel_spmd

    x = np.asarray(x)
    weight = np.asarray(weight)
    bias = np.asarray(bias)
    nc = _build()
    in_maps = _host_prep(x, weight, bias)
    res = run_bass_kernel_spmd(nc, in_maps, core_ids=list(range(N_CORES)))
    out = np.concatenate(
        [
            res.results[r]["y"].astype(np.float32).reshape(BPC, COUT, H, W)
            for r in range(N_CORES)
        ],
        axis=0,
    )
    return out


# revision 27
# speedup vs baseline: 1.0668x; 1.0668x over previous
"""FFTConv2d kernel for trn2, 8 NeuronCores.

Math: reference einsum 'bchw,oihw->bohw' factorizes:
  Y[b,o] = conv_full(sum_c x[b,c], sum_i w[o,i])[1:-1,1:-1] + bias[o]
i.e. a single-channel 3x3 "same" convolution (flipped kernel) per (b,o).

v5: fp16 end-to-end (PSUM fp32), uneven row-slices [8,40,40,40], and an
HBM round-trip to build the conv operand (only 6 DMAs per slice, no
SBUF->SBUF traffic). Per core (2 batches), per slice:
  1. DMA x slice in as fp16, partitions=(b,c), SH+2 row-slots with memset
     zero-rows at the image edges.
  2. Channel-sum matmul (ones lhsT), 4x col-tiled via tile_position
     (0,32g): phase g covers SH/4 output rows; psum partitions 32g+b.
  3. Copy PSUM -> phase-split staging fp16 (row stride 130, pad cols
     memset once per buffer), one [128,512] copy per 4-slot chunk.
  4. S-out: 2 DMAs (per b) write the padded channel-sum image to an HBM
     scratch slab, merging the 4 phases via overlapping halo writes.
  5. P9-in: 2 DMAs (per b) read back P9 [19, SH*130]: flat HBM src AP
     [[130,3],[1,3],[1,PWIN]] encodes both 3x3 shifts; P9 partition
     9b + 3(2-u) + (2-v). Partition 18 holds ones (bias row).
  6. Conv: per <=3-row chunk one fp16 matmul wb[19,128].T @ P9 window ->
     PSUM [128,3,130].
  7. Copy PSUM -> yt fp16 (drop pad cols), DMA yt -> HBM; host upcasts.
Conv trails channel-sum by 2 slices (DEPTH=2).
"""

import os
import sys
from functools import lru_cache

import numpy as np

for _p in ("/opt/trn_rl_repo", "/root/.axon_site/_ro/trn_rl_repo"):
    if os.path.isdir(_p) and _p not in sys.path:
        sys.path.insert(0, _p)

import ml_dtypes

B, CIN, COUT, H, W = 16, 64, 64, 128, 128
N_CORES = 8
BPC = B // N_CORES  # batches per core = 2
WROW = W + 2  # padded row stride = 130
NPH = 4  # col-tile phases per slice
NPART = BPC * CIN  # 128 input partitions (b, c)
NOUT = BPC * COUT  # 128 output partitions (b, o)
KCONV = BPC * 9 + 1  # 19 conv contraction rows

SLICE_SH = [8, 48, 48, 24]  # output rows per slice
NS = len(SLICE_SH)
SH_MAX = max(SLICE_SH)
PSLOT_MAX = SH_MAX // NPH + 2  # 16
PHLEN = PSLOT_MAX * WROW + 2  # staging cols = 1562
ST2 = (SH_MAX + 2) * WROW + 2  # HBM scratch cols per (b, slice) = 5462
PWIN_MAX = SH_MAX * WROW  # 5200

_SLICE_R0 = np.cumsum([0] + SLICE_SH).tolist()  # row starts
_SLICE_ROWS = []  # clipped input rows
_SLICE_OFF = []
_off = 0
for _s in range(NS):
    _h0 = max(0, _SLICE_R0[_s] - 1)
    _he = min(H, _SLICE_R0[_s + 1] + 1)
    _SLICE_ROWS.append((_h0, _he))
    _SLICE_OFF.append(_off)
    _off += (_he - _h0) * W
XPACK_LEN = _off


@lru_cache(maxsize=1)
def _build():
    import concourse.bacc as bacc
    import concourse.mybir as mybir
    import concourse.tile as tile
    from concourse.ap import AP

    f32 = mybir.dt.float32
    f16 = mybir.dt.float16

    nc = bacc.Bacc("TRN2", target_bir_lowering=False, debug=False, num_devices=N_CORES)

    xp = nc.dram_tensor("xpack", [NPART, XPACK_LEN], f16, kind="ExternalInput")
    ones_cs = nc.dram_tensor("ones_cs", [NPART, BPC], f16, kind="ExternalInput")
    wb = nc.dram_tensor("wb", [KCONV, NOUT], f16, kind="ExternalInput")
    ones_p = nc.dram_tensor("ones_p", [1, PWIN_MAX], f16, kind="ExternalInput")
    y = nc.dram_tensor("y", [NOUT, H * W], f16, kind="ExternalOutput")
    # HBM scratch: padded channel-sum image per (b, slice)
    ssc = nc.dram_tensor("ssc", [1, BPC * NS * ST2], f16, kind="Internal")

    with tile.TileContext(nc) as tc:
        with (
            tc.tile_pool(name="xin", bufs=4) as xin_pool,
            tc.tile_pool(name="sp", bufs=1) as sp_pool,
            tc.tile_pool(name="pbuf", bufs=1) as p_pool,
            tc.tile_pool(name="yout", bufs=2) as y_pool,
            tc.tile_pool(name="consts", bufs=1) as c_pool,
            tc.tile_pool(name="cs_ps", bufs=1, space="PSUM") as cs_psum,
            tc.tile_pool(name="cv_ps", bufs=4, space="PSUM") as cv_psum,
        ):
            def emit_in(s):
                h0, he = _SLICE_ROWS[s]
                ncols = (he - h0) * W
                nslot = SLICE_SH[s] + 2
                xin = xin_pool.tile([NPART, nslot * W], f16, tag="xin")
                o = _SLICE_OFF[s]
                d0 = (h0 - (_SLICE_R0[s] - 1)) * W  # W for s=0 else 0
                if s == 0:
                    nc.vector.memset(xin[:, 0:W], 0.0)
                if s == NS - 1:
                    nc.vector.memset(xin[:, d0 + ncols :], 0.0)
                nc.scalar.dma_start(
                    out=xin[:, d0 : d0 + ncols], in_=xp.ap()[:, o : o + ncols]
                )
                return xin

            # slice-0 input first for the fastest pipeline start
            xins = {}
            xins[0] = emit_in(0)
            ones_t = c_pool.tile([NPART, BPC], f16, tag="ones_cs")
            nc.scalar.dma_start(out=ones_t[:, :], in_=ones_cs.ap()[:, :])
            wb_t = c_pool.tile([KCONV, NOUT], f16, tag="wb")
            nc.scalar.dma_start(out=wb_t[:, :], in_=wb.ap()[:, :])
            for s in range(1, NS):
                xins[s] = emit_in(s)

            NBUF = 2
            NBUF9 = 3
            spbufs = []
            p9bufs = []
            for pi in range(NBUF):
                sp = sp_pool.tile([NPART, PHLEN], f16, tag=f"SP{pi}")
                spt = sp.tensor
                nc.vector.memset(sp[:, 0:1], 0.0)
                nc.vector.memset(
                    AP(tensor=spt, offset=WROW - 1,
                       ap=[[PHLEN, NPART], [WROW, PSLOT_MAX], [1, 2]]),
                    0.0,
                )
                nc.vector.memset(sp[:, PHLEN - 1 : PHLEN], 0.0)
                spbufs.append(sp)
            for pi in range(NBUF9):
                p9 = p_pool.tile([KCONV, PWIN_MAX], f16, tag=f"P9{pi}")
                nc.sync.dma_start(
                    out=p9[KCONV - 1 : KCONV, :], in_=ones_p.ap()[0:1, :]
                )
                p9bufs.append(p9)

            def cs_thunks(s, xin):
                sh = SLICE_SH[s]
                prows = sh // NPH
                pslot = prows + 2
                nchunk = (pslot + 3) // 4
                sp = spbufs[s % NBUF]
                spt = sp.tensor
                p9 = p9bufs[s % NBUF9]

                def chunk(cb):
                    sl0 = 4 * cb
                    nrows = min(4, pslot - sl0)
                    n = nrows * W
                    ps = cs_psum.tile([NPART, 4, W], f32, tag=f"cs{cb}",
                                      name=f"cs{s}_{cb}")
                    for g in range(NPH):
                        c0 = (g * prows + sl0) * W
                        nc.tensor.matmul(
                            ps[32 * g : 32 * g + BPC, :nrows, :],
                            ones_t[:, :],
                            xin[:, c0 : c0 + n],
                            start=True,
                            stop=True,
                            tile_position=(0, 32 * g),
                        )
                    dst = AP(
                        tensor=spt,
                        offset=sl0 * WROW + 1,
                        ap=[[PHLEN, NPART], [WROW, nrows], [1, W]],
                    )
                    if cb == 1:
                        nc.vector.tensor_copy(dst, ps[:, :nrows, :])
                    else:
                        nc.scalar.copy(dst, ps[:, :nrows, :])

                def tail():
                    # S-out then P9-in via HBM scratch
                    pwin = sh * WROW
                    for b in range(BPC):
                        base = (b * NS + s) * ST2
                        nc.gpsimd.dma_start(
                            out=AP(
                                tensor=ssc,
                                offset=base,
                                ap=[[prows * WROW, NPH], [1, pslot * WROW]],
                            ),
                            in_=AP(
                                tensor=spt,
                                offset=b * PHLEN,
                                ap=[[32 * PHLEN, NPH], [1, pslot * WROW]],
                            ),
                        )
                    for b in range(BPC):
                        base = (b * NS + s) * ST2
                        nc.sync.dma_start(
                            out=p9[9 * b : 9 * b + 9, 0:pwin],
                            in_=AP(
                                tensor=ssc,
                                offset=base,
                                ap=[[WROW, 3], [1, 3], [1, pwin]],
                            ),
                        )

                return [lambda cb=cb: chunk(cb) for cb in range(nchunk)] + [tail], p9

            def cv_thunks(s, p9):
                sh = SLICE_SH[s]
                r0 = _SLICE_R0[s]
                yt = y_pool.tile([NOUT, sh, W], f16, tag="yout",
                                 name=f"yt{s}")
                nchunk = (sh + 2) // 3
                half = (nchunk + 1) // 2
                hrow = half * 3

                def chunk(c):
                    rr0 = c * 3
                    nrr = min(3, sh - rr0)
                    ps = cv_psum.tile([NOUT, 3, WROW], f32, tag="cv",
                                      name=f"cv{s}_{c}")
                    nc.tensor.matmul(
                        ps[:, :nrr, :],
                        wb_t[:, :],
                        p9[:, rr0 * WROW : (rr0 + nrr) * WROW],
                        start=True,
                        stop=True,
                    )
                    if c % 2 == 0:
                        nc.vector.tensor_copy(
                            yt[:, rr0 : rr0 + nrr, :], ps[:, :nrr, 0:W]
                        )
                    else:
                        nc.scalar.copy(yt[:, rr0 : rr0 + nrr, :],
                                       ps[:, :nrr, 0:W])
                    if sh > 8 and c == half - 1:
                        nc.gpsimd.dma_start(
                            out=y.ap()[:, r0 * W : (r0 + hrow) * W],
                            in_=yt[:, :hrow, :],
                        )
                    if c == nchunk - 1:
                        a = hrow if sh > 8 else 0
                        nc.gpsimd.dma_start(
                            out=y.ap()[:, (r0 + a) * W : (r0 + sh) * W],
                            in_=yt[:, a:, :],
                        )

                return [lambda c=c: chunk(c) for c in range(nchunk)]

            DEPTH = 2
            p9s = {}
            for s in range(NS + DEPTH):
                if s < NS:
                    A, p9s[s] = cs_thunks(s, xins[s])
                    for t in A:
                        t()
                if s >= DEPTH:
                    for t in cv_thunks(s - DEPTH, p9s[s - DEPTH]):
                        t()

    nc.compile()
    return nc


def _host_prep(x, weight, bias):
    fh = np.float16
    wsum = weight.sum(axis=1)  # [COUT, 3, 3] fp32
    wb = np.zeros((KCONV, NOUT), np.float32)
    for b in range(BPC):
        for u in range(3):
            for v in range(3):
                wb[9 * b + 3 * (2 - u) + (2 - v),
                   b * COUT : (b + 1) * COUT] = wsum[:, u, v]
    wb[KCONV - 1, :] = np.tile(bias, BPC)
    wb = wb.astype(fh)
    ones_cs = np.zeros((NPART, BPC), np.float32)
    for b in range(BPC):
        ones_cs[b * CIN : (b + 1) * CIN, b] = 1.0
    ones_cs = ones_cs.astype(fh)
    ones_p = np.ones((1, PWIN_MAX), dtype=fh)

    in_maps = []
    for r in range(N_CORES):
        xs = np.ascontiguousarray(
            x[r * BPC : (r + 1) * BPC].reshape(NPART, H, W)
        ).astype(fh)
        xpack = np.empty((NPART, XPACK_LEN), dtype=fh)
        for s in range(NS):
            h0, he = _SLICE_ROWS[s]
            n = (he - h0) * W
            o = _SLICE_OFF[s]
            xpack[:, o : o + n] = xs[:, h0:he].reshape(NPART, n)
        in_maps.append(
            {
                "xpack": xpack,
                "ones_cs": ones_cs,
                "wb": wb,
                "ones_p": ones_p,
            }
        )
    return in_maps


def kernel(x, weight, bias):
    from concourse.bass_utils import run_bass_kernel_spmd

    x = np.asarray(x)
    weight = np.asarray(weight)
    bias = np.asarray(bias)
    nc = _build()
    in_maps = _host_prep(x, weight, bias)
    res = run_bass_kernel_spmd(nc, in_maps, core_ids=list(range(N_CORES)))
    out = np.concatenate(
        [
            res.results[r]["y"].astype(np.float32).reshape(BPC, COUT, H, W)
            for r in range(N_CORES)
        ],
        axis=0,
    )
    return out


# revision 28
# speedup vs baseline: 1.1183x; 1.0482x over previous
"""FFTConv2d kernel for trn2, 8 NeuronCores.

Math: reference einsum 'bchw,oihw->bohw' factorizes:
  Y[b,o] = conv_full(sum_c x[b,c], sum_i w[o,i])[1:-1,1:-1] + bias[o]
i.e. a single-channel 3x3 "same" convolution (flipped kernel) per (b,o).

v5: fp16 end-to-end (PSUM fp32), uneven row-slices [8,40,40,40], and an
HBM round-trip to build the conv operand (only 6 DMAs per slice, no
SBUF->SBUF traffic). Per core (2 batches), per slice:
  1. DMA x slice in as fp16, partitions=(b,c), SH+2 row-slots with memset
     zero-rows at the image edges.
  2. Channel-sum matmul (ones lhsT), 4x col-tiled via tile_position
     (0,32g): phase g covers SH/4 output rows; psum partitions 32g+b.
  3. Copy PSUM -> phase-split staging fp16 (row stride 130, pad cols
     memset once per buffer), one [128,512] copy per 4-slot chunk.
  4. S-out: 2 DMAs (per b) write the padded channel-sum image to an HBM
     scratch slab, merging the 4 phases via overlapping halo writes.
  5. P9-in: 2 DMAs (per b) read back P9 [19, SH*130]: flat HBM src AP
     [[130,3],[1,3],[1,PWIN]] encodes both 3x3 shifts; P9 partition
     9b + 3(2-u) + (2-v). Partition 18 holds ones (bias row).
  6. Conv: per <=3-row chunk one fp16 matmul wb[19,128].T @ P9 window ->
     PSUM [128,3,130].
  7. Copy PSUM -> yt fp16 (drop pad cols), DMA yt -> HBM; host upcasts.
Conv trails channel-sum by 2 slices (DEPTH=2).
"""

import os
import sys
from functools import lru_cache

import numpy as np

for _p in ("/opt/trn_rl_repo", "/root/.axon_site/_ro/trn_rl_repo"):
    if os.path.isdir(_p) and _p not in sys.path:
        sys.path.insert(0, _p)

import ml_dtypes

B, CIN, COUT, H, W = 16, 64, 64, 128, 128
N_CORES = 8
BPC = B // N_CORES  # batches per core = 2
WROW = W + 2  # padded row stride = 130
NPH = 4  # col-tile phases per slice
NPART = BPC * CIN  # 128 input partitions (b, c)
NOUT = BPC * COUT  # 128 output partitions (b, o)
KCONV = BPC * 9 + 1  # 19 conv contraction rows

SLICE_SH = [8, 48, 48, 24]  # output rows per slice
NS = len(SLICE_SH)
SH_MAX = max(SLICE_SH)
PSLOT_MAX = SH_MAX // NPH + 2  # 16
PHLEN = PSLOT_MAX * WROW + 2  # staging cols = 1562
ST2 = (SH_MAX + 2) * WROW + 2  # HBM scratch cols per (b, slice) = 5462
PWIN_MAX = SH_MAX * WROW  # 5200

_SLICE_R0 = np.cumsum([0] + SLICE_SH).tolist()  # row starts
_SLICE_ROWS = []  # clipped input rows
_SLICE_OFF = []
_off = 0
for _s in range(NS):
    _h0 = max(0, _SLICE_R0[_s] - 1)
    _he = min(H, _SLICE_R0[_s + 1] + 1)
    _SLICE_ROWS.append((_h0, _he))
    _SLICE_OFF.append(_off)
    _off += (_he - _h0) * W
XPACK_LEN = _off


@lru_cache(maxsize=1)
def _build():
    import concourse.bacc as bacc
    import concourse.mybir as mybir
    import concourse.tile as tile
    from concourse.ap import AP

    f32 = mybir.dt.float32
    f16 = mybir.dt.float16

    nc = bacc.Bacc("TRN2", target_bir_lowering=False, debug=False, num_devices=N_CORES)

    xp = nc.dram_tensor("xpack", [NPART, XPACK_LEN], f16, kind="ExternalInput")
    ones_cs = nc.dram_tensor("ones_cs", [NPART, BPC], f16, kind="ExternalInput")
    wb = nc.dram_tensor("wb", [KCONV, NOUT], f16, kind="ExternalInput")
    ones_p = nc.dram_tensor("ones_p", [1, PWIN_MAX], f16, kind="ExternalInput")
    y = nc.dram_tensor("y", [NOUT, H * W], f16, kind="ExternalOutput")
    # HBM scratch: padded channel-sum image per (b, slice)
    ssc = nc.dram_tensor("ssc", [1, BPC * NS * ST2], f16, kind="Internal")

    with tile.TileContext(nc) as tc:
        with (
            tc.tile_pool(name="xin", bufs=4) as xin_pool,
            tc.tile_pool(name="sp", bufs=1) as sp_pool,
            tc.tile_pool(name="pbuf", bufs=1) as p_pool,
            tc.tile_pool(name="yout", bufs=2) as y_pool,
            tc.tile_pool(name="consts", bufs=1) as c_pool,
            tc.tile_pool(name="cs_ps", bufs=1, space="PSUM") as cs_psum,
            tc.tile_pool(name="cv_ps", bufs=4, space="PSUM") as cv_psum,
        ):
            def emit_in(s):
                h0, he = _SLICE_ROWS[s]
                ncols = (he - h0) * W
                nslot = SLICE_SH[s] + 2
                xin = xin_pool.tile([NPART, nslot * W], f16, tag="xin")
                o = _SLICE_OFF[s]
                d0 = (h0 - (_SLICE_R0[s] - 1)) * W  # W for s=0 else 0
                if s == 0:
                    nc.vector.memset(xin[:, 0:W], 0.0)
                if s == NS - 1:
                    nc.vector.memset(xin[:, d0 + ncols :], 0.0)
                nc.scalar.dma_start(
                    out=xin[:, d0 : d0 + ncols], in_=xp.ap()[:, o : o + ncols]
                )
                return xin

            # slice-0 input first for the fastest pipeline start
            xins = {}
            xins[0] = emit_in(0)
            ones_t = c_pool.tile([NPART, BPC], f16, tag="ones_cs")
            nc.scalar.dma_start(out=ones_t[:, :], in_=ones_cs.ap()[:, :])
            wb_t = c_pool.tile([KCONV, NOUT], f16, tag="wb")
            nc.scalar.dma_start(out=wb_t[:, :], in_=wb.ap()[:, :])
            for s in range(1, NS):
                xins[s] = emit_in(s)

            NBUF = 2
            NBUF9 = 3
            spbufs = []
            p9bufs = []
            for pi in range(NBUF):
                sp = sp_pool.tile([NPART, PHLEN], f16, tag=f"SP{pi}")
                spt = sp.tensor
                nc.vector.memset(sp[:, 0:1], 0.0)
                nc.vector.memset(
                    AP(tensor=spt, offset=WROW - 1,
                       ap=[[PHLEN, NPART], [WROW, PSLOT_MAX], [1, 2]]),
                    0.0,
                )
                nc.vector.memset(sp[:, PHLEN - 1 : PHLEN], 0.0)
                spbufs.append(sp)
            for pi in range(NBUF9):
                p9 = p_pool.tile([KCONV, PWIN_MAX], f16, tag=f"P9{pi}")
                nc.sync.dma_start(
                    out=p9[KCONV - 1 : KCONV, :], in_=ones_p.ap()[0:1, :]
                )
                p9bufs.append(p9)

            def cs_thunks(s, xin):
                sh = SLICE_SH[s]
                prows = sh // NPH
                pslot = prows + 2
                nchunk = (pslot + 3) // 4
                sp = spbufs[s % NBUF]
                spt = sp.tensor
                p9 = p9bufs[s % NBUF9]

                def chunk(cb):
                    sl0 = 4 * cb
                    nrows = min(4, pslot - sl0)
                    n = nrows * W
                    ps = cs_psum.tile([NPART, 4, W], f32, tag=f"cs{cb}",
                                      name=f"cs{s}_{cb}")
                    for g in range(NPH):
                        c0 = (g * prows + sl0) * W
                        nc.tensor.matmul(
                            ps[32 * g : 32 * g + BPC, :nrows, :],
                            ones_t[:, :],
                            xin[:, c0 : c0 + n],
                            start=True,
                            stop=True,
                            tile_position=(0, 32 * g),
                        )
                    dst = AP(
                        tensor=spt,
                        offset=sl0 * WROW + 1,
                        ap=[[PHLEN, NPART], [WROW, nrows], [1, W]],
                    )
                    nc.any.tensor_copy(dst, ps[:, :nrows, :])

                def tail():
                    # S-out then P9-in via HBM scratch
                    pwin = sh * WROW
                    for b in range(BPC):
                        base = (b * NS + s) * ST2
                        nc.gpsimd.dma_start(
                            out=AP(
                                tensor=ssc,
                                offset=base,
                                ap=[[prows * WROW, NPH], [1, pslot * WROW]],
                            ),
                            in_=AP(
                                tensor=spt,
                                offset=b * PHLEN,
                                ap=[[32 * PHLEN, NPH], [1, pslot * WROW]],
                            ),
                        )
                    for b in range(BPC):
                        base = (b * NS + s) * ST2
                        nc.sync.dma_start(
                            out=p9[9 * b : 9 * b + 9, 0:pwin],
                            in_=AP(
                                tensor=ssc,
                                offset=base,
                                ap=[[WROW, 3], [1, 3], [1, pwin]],
                            ),
                        )

                return [lambda cb=cb: chunk(cb) for cb in range(nchunk)] + [tail], p9

            def cv_thunks(s, p9):
                sh = SLICE_SH[s]
                r0 = _SLICE_R0[s]
                yt = y_pool.tile([NOUT, sh, W], f16, tag="yout",
                                 name=f"yt{s}")
                nchunk = (sh + 2) // 3
                half = (nchunk + 1) // 2
                hrow = half * 3

                def chunk(c):
                    rr0 = c * 3
                    nrr = min(3, sh - rr0)
                    ps = cv_psum.tile([NOUT, 3, WROW], f32, tag="cv",
                                      name=f"cv{s}_{c}")
                    nc.tensor.matmul(
                        ps[:, :nrr, :],
                        wb_t[:, :],
                        p9[:, rr0 * WROW : (rr0 + nrr) * WROW],
                        start=True,
                        stop=True,
                    )
                    nc.any.tensor_copy(yt[:, rr0 : rr0 + nrr, :],
                                       ps[:, :nrr, 0:W])
                    if sh > 8 and c == half - 1:
                        nc.gpsimd.dma_start(
                            out=y.ap()[:, r0 * W : (r0 + hrow) * W],
                            in_=yt[:, :hrow, :],
                        )
                    if c == nchunk - 1:
                        a = hrow if sh > 8 else 0
                        nc.gpsimd.dma_start(
                            out=y.ap()[:, (r0 + a) * W : (r0 + sh) * W],
                            in_=yt[:, a:, :],
                        )

                return [lambda c=c: chunk(c) for c in range(nchunk)]

            DEPTH = 2
            p9s = {}
            for s in range(NS + DEPTH):
                if s < NS:
                    A, p9s[s] = cs_thunks(s, xins[s])
                    for t in A:
                        t()
                if s >= DEPTH:
                    for t in cv_thunks(s - DEPTH, p9s[s - DEPTH]):
                        t()

    nc.compile()
    return nc


def _host_prep(x, weight, bias):
    fh = np.float16
    wsum = weight.sum(axis=1)  # [COUT, 3, 3] fp32
    wb = np.zeros((KCONV, NOUT), np.float32)
    for b in range(BPC):
        for u in range(3):
            for v in range(3):
                wb[9 * b + 3 * (2 - u) + (2 - v),
                   b * COUT : (b + 1) * COUT] = wsum[:, u, v]
    wb[KCONV - 1, :] = np.tile(bias, BPC)
    wb = wb.astype(fh)
    ones_cs = np.zeros((NPART, BPC), np.float32)
    for b in range(BPC):
        ones_cs[b * CIN : (b + 1) * CIN, b] = 1.0
    ones_cs = ones_cs.astype(fh)
    ones_p = np.ones((1, PWIN_MAX), dtype=fh)

    in_maps = []
    for r in range(N_CORES):
        xs = np.ascontiguousarray(
            x[r * BPC : (r + 1) * BPC].reshape(NPART, H, W)
        ).astype(fh)
        xpack = np.empty((NPART, XPACK_LEN), dtype=fh)
        for s in range(NS):
            h0, he = _SLICE_ROWS[s]
            n = (he - h0) * W
            o = _SLICE_OFF[s]
            xpack[:, o : o + n] = xs[:, h0:he].reshape(NPART, n)
        in_maps.append(
            {
                "xpack": xpack,
                "ones_cs": ones_cs,
                "wb": wb,
                "ones_p": ones_p,
            }
        )
    return in_maps


def kernel(x, weight, bias):
    from concourse.bass_utils import run_bass_kernel_spmd

    x = np.asarray(x)
    weight = np.asarray(weight)
    bias = np.asarray(bias)
    nc = _build()
    in_maps = _host_prep(x, weight, bias)
    res = run_bass_kernel_spmd(nc, in_maps, core_ids=list(range(N_CORES)))
    out = np.concatenate(
        [
            res.results[r]["y"].astype(np.float32).reshape(BPC, COUT, H, W)
            for r in range(N_CORES)
        ],
        axis=0,
    )
    return out


# revision 29
# speedup vs baseline: 1.1250x; 1.0061x over previous
"""FFTConv2d kernel for trn2, 8 NeuronCores.

Math: reference einsum 'bchw,oihw->bohw' factorizes:
  Y[b,o] = conv_full(sum_c x[b,c], sum_i w[o,i])[1:-1,1:-1] + bias[o]
i.e. a single-channel 3x3 "same" convolution (flipped kernel) per (b,o).

v5: fp16 end-to-end (PSUM fp32), uneven row-slices [8,40,40,40], and an
HBM round-trip to build the conv operand (only 6 DMAs per slice, no
SBUF->SBUF traffic). Per core (2 batches), per slice:
  1. DMA x slice in as fp16, partitions=(b,c), SH+2 row-slots with memset
     zero-rows at the image edges.
  2. Channel-sum matmul (ones lhsT), 4x col-tiled via tile_position
     (0,32g): phase g covers SH/4 output rows; psum partitions 32g+b.
  3. Copy PSUM -> phase-split staging fp16 (row stride 130, pad cols
     memset once per buffer), one [128,512] copy per 4-slot chunk.
  4. S-out: 2 DMAs (per b) write the padded channel-sum image to an HBM
     scratch slab, merging the 4 phases via overlapping halo writes.
  5. P9-in: 2 DMAs (per b) read back P9 [19, SH*130]: flat HBM src AP
     [[130,3],[1,3],[1,PWIN]] encodes both 3x3 shifts; P9 partition
     9b + 3(2-u) + (2-v). Partition 18 holds ones (bias row).
  6. Conv: per <=3-row chunk one fp16 matmul wb[19,128].T @ P9 window ->
     PSUM [128,3,130].
  7. Copy PSUM -> yt fp16 (drop pad cols), DMA yt -> HBM; host upcasts.
Conv trails channel-sum by 2 slices (DEPTH=2).
"""

import os
import sys
from functools import lru_cache

import numpy as np

for _p in ("/opt/trn_rl_repo", "/root/.axon_site/_ro/trn_rl_repo"):
    if os.path.isdir(_p) and _p not in sys.path:
        sys.path.insert(0, _p)

import ml_dtypes

B, CIN, COUT, H, W = 16, 64, 64, 128, 128
N_CORES = 8
BPC = B // N_CORES  # batches per core = 2
WROW = W + 2  # padded row stride = 130
NPH = 4  # col-tile phases per slice
NPART = BPC * CIN  # 128 input partitions (b, c)
NOUT = BPC * COUT  # 128 output partitions (b, o)
KCONV = BPC * 9 + 1  # 19 conv contraction rows

SLICE_SH = [8, 48, 48, 24]  # output rows per slice
NS = len(SLICE_SH)
SH_MAX = max(SLICE_SH)
PSLOT_MAX = SH_MAX // NPH + 2  # 16
PHLEN = PSLOT_MAX * WROW + 2  # staging cols = 1562
ST2 = (SH_MAX + 2) * WROW + 2  # HBM scratch cols per (b, slice) = 5462
PWIN_MAX = SH_MAX * WROW  # 5200

_SLICE_R0 = np.cumsum([0] + SLICE_SH).tolist()  # row starts
_SLICE_ROWS = []  # clipped input rows
_SLICE_OFF = []
_off = 0
for _s in range(NS):
    _h0 = max(0, _SLICE_R0[_s] - 1)
    _he = min(H, _SLICE_R0[_s + 1] + 1)
    _SLICE_ROWS.append((_h0, _he))
    _SLICE_OFF.append(_off)
    _off += (_he - _h0) * W
XPACK_LEN = _off


@lru_cache(maxsize=1)
def _build():
    import concourse.bacc as bacc
    import concourse.mybir as mybir
    import concourse.tile as tile
    from concourse.ap import AP

    f32 = mybir.dt.float32
    f16 = mybir.dt.float16

    nc = bacc.Bacc("TRN2", target_bir_lowering=False, debug=False, num_devices=N_CORES)

    xp = nc.dram_tensor("xpack", [NPART, XPACK_LEN], f16, kind="ExternalInput")
    ones_cs = nc.dram_tensor("ones_cs", [NPART, BPC], f16, kind="ExternalInput")
    wb = nc.dram_tensor("wb", [KCONV, NOUT], f16, kind="ExternalInput")
    ones_p = nc.dram_tensor("ones_p", [1, PWIN_MAX], f16, kind="ExternalInput")
    y = nc.dram_tensor("y", [NOUT, H * W], f16, kind="ExternalOutput")
    # HBM scratch: padded channel-sum image per (b, slice)
    ssc = nc.dram_tensor("ssc", [1, BPC * NS * ST2], f16, kind="Internal")

    with tile.TileContext(nc) as tc:
        with (
            tc.tile_pool(name="xin", bufs=4) as xin_pool,
            tc.tile_pool(name="sp", bufs=1) as sp_pool,
            tc.tile_pool(name="pbuf", bufs=1) as p_pool,
            tc.tile_pool(name="yout", bufs=2) as y_pool,
            tc.tile_pool(name="consts", bufs=1) as c_pool,
            tc.tile_pool(name="cs_ps", bufs=1, space="PSUM") as cs_psum,
            tc.tile_pool(name="cv_ps", bufs=4, space="PSUM") as cv_psum,
        ):
            def emit_in(s):
                h0, he = _SLICE_ROWS[s]
                ncols = (he - h0) * W
                nslot = SLICE_SH[s] + 2
                xin = xin_pool.tile([NPART, nslot * W], f16, tag="xin")
                o = _SLICE_OFF[s]
                d0 = (h0 - (_SLICE_R0[s] - 1)) * W  # W for s=0 else 0
                if s == 0:
                    nc.vector.memset(xin[:, 0:W], 0.0)
                if s == NS - 1:
                    nc.vector.memset(xin[:, d0 + ncols :], 0.0)
                nc.scalar.dma_start(
                    out=xin[:, d0 : d0 + ncols], in_=xp.ap()[:, o : o + ncols]
                )
                return xin

            # slice-0 input first for the fastest pipeline start
            xins = {}
            xins[0] = emit_in(0)
            ones_t = c_pool.tile([NPART, BPC], f16, tag="ones_cs")
            nc.scalar.dma_start(out=ones_t[:, :], in_=ones_cs.ap()[:, :])
            wb_t = c_pool.tile([KCONV, NOUT], f16, tag="wb")
            nc.scalar.dma_start(out=wb_t[:, :], in_=wb.ap()[:, :])
            for s in range(1, NS):
                xins[s] = emit_in(s)

            NBUF = 2
            NBUF9 = 3
            spbufs = []
            p9bufs = []
            for pi in range(NBUF):
                sp = sp_pool.tile([NPART, PHLEN], f16, tag=f"SP{pi}")
                spt = sp.tensor
                nc.vector.memset(sp[:, 0:1], 0.0)
                nc.vector.memset(
                    AP(tensor=spt, offset=WROW - 1,
                       ap=[[PHLEN, NPART], [WROW, PSLOT_MAX], [1, 2]]),
                    0.0,
                )
                nc.vector.memset(sp[:, PHLEN - 1 : PHLEN], 0.0)
                spbufs.append(sp)
            for pi in range(NBUF9):
                p9 = p_pool.tile([KCONV, PWIN_MAX], f16, tag=f"P9{pi}")
                nc.sync.dma_start(
                    out=p9[KCONV - 1 : KCONV, :], in_=ones_p.ap()[0:1, :]
                )
                p9bufs.append(p9)

            def cs_thunks(s, xin):
                sh = SLICE_SH[s]
                prows = sh // NPH
                pslot = prows + 2
                nchunk = (pslot + 3) // 4
                sp = spbufs[s % NBUF]
                spt = sp.tensor
                p9 = p9bufs[s % NBUF9]

                def chunk(cb):
                    sl0 = 4 * cb
                    nrows = min(4, pslot - sl0)
                    n = nrows * W
                    ps = cs_psum.tile([NPART, 4, W], f32, tag=f"cs{cb}",
                                      name=f"cs{s}_{cb}")
                    for g in range(NPH):
                        c0 = (g * prows + sl0) * W
                        nc.tensor.matmul(
                            ps[32 * g : 32 * g + BPC, :nrows, :],
                            ones_t[:, :],
                            xin[:, c0 : c0 + n],
                            start=True,
                            stop=True,
                            tile_position=(0, 32 * g),
                        )
                    dst = AP(
                        tensor=spt,
                        offset=sl0 * WROW + 1,
                        ap=[[PHLEN, NPART], [WROW, nrows], [1, W]],
                    )
                    nc.any.tensor_copy(dst, ps[:, :nrows, :])

                def tail():
                    # S-out then P9-in via HBM scratch
                    pwin = sh * WROW
                    for b in range(BPC):
                        base = (b * NS + s) * ST2
                        nc.gpsimd.dma_start(
                            out=AP(
                                tensor=ssc,
                                offset=base,
                                ap=[[prows * WROW, NPH], [1, pslot * WROW]],
                            ),
                            in_=AP(
                                tensor=spt,
                                offset=b * PHLEN,
                                ap=[[32 * PHLEN, NPH], [1, pslot * WROW]],
                            ),
                        )
                    for b in range(BPC):
                        base = (b * NS + s) * ST2
                        nc.sync.dma_start(
                            out=p9[9 * b : 9 * b + 9, 0:pwin],
                            in_=AP(
                                tensor=ssc,
                                offset=base,
                                ap=[[WROW, 3], [1, 3], [1, pwin]],
                            ),
                        )

                return [lambda cb=cb: chunk(cb) for cb in range(nchunk)] + [tail], p9

            def cv_thunks(s, p9):
                sh = SLICE_SH[s]
                r0 = _SLICE_R0[s]
                p9t = p9.tensor
                yt = y_pool.tile([NOUT, sh, W], f16, tag="yout",
                                 name=f"yt{s}")
                nchunk = sh // 4
                half = (nchunk + 1) // 2
                hrow = half * 4

                def chunk(c):
                    rr0 = c * 4
                    ps = cv_psum.tile([NOUT, 4, W], f32, tag="cv",
                                      name=f"cv{s}_{c}")
                    nc.tensor.matmul(
                        ps[:, :, :],
                        wb_t[:, :],
                        AP(tensor=p9t, offset=rr0 * WROW,
                           ap=[[PWIN_MAX, KCONV], [WROW, 4], [1, W]]),
                        start=True,
                        stop=True,
                    )
                    nc.any.tensor_copy(yt[:, rr0 : rr0 + 4, :], ps[:, :, :])
                    if sh > 8 and c == half - 1:
                        nc.gpsimd.dma_start(
                            out=y.ap()[:, r0 * W : (r0 + hrow) * W],
                            in_=yt[:, :hrow, :],
                        )
                    if c == nchunk - 1:
                        a = hrow if sh > 8 else 0
                        nc.gpsimd.dma_start(
                            out=y.ap()[:, (r0 + a) * W : (r0 + sh) * W],
                            in_=yt[:, a:, :],
                        )

                return [lambda c=c: chunk(c) for c in range(nchunk)]

            DEPTH = 2
            p9s = {}
            for s in range(NS + DEPTH):
                if s < NS:
                    A, p9s[s] = cs_thunks(s, xins[s])
                    for t in A:
                        t()
                if s >= DEPTH:
                    for t in cv_thunks(s - DEPTH, p9s[s - DEPTH]):
                        t()

    nc.compile()
    return nc


def _host_prep(x, weight, bias):
    fh = np.float16
    wsum = weight.sum(axis=1)  # [COUT, 3, 3] fp32
    wb = np.zeros((KCONV, NOUT), np.float32)
    for b in range(BPC):
        for u in range(3):
            for v in range(3):
                wb[9 * b + 3 * (2 - u) + (2 - v),
                   b * COUT : (b + 1) * COUT] = wsum[:, u, v]
    wb[KCONV - 1, :] = np.tile(bias, BPC)
    wb = wb.astype(fh)
    ones_cs = np.zeros((NPART, BPC), np.float32)
    for b in range(BPC):
        ones_cs[b * CIN : (b + 1) * CIN, b] = 1.0
    ones_cs = ones_cs.astype(fh)
    ones_p = np.ones((1, PWIN_MAX), dtype=fh)

    in_maps = []
    for r in range(N_CORES):
        xs = np.ascontiguousarray(
            x[r * BPC : (r + 1) * BPC].reshape(NPART, H, W)
        ).astype(fh)
        xpack = np.empty((NPART, XPACK_LEN), dtype=fh)
        for s in range(NS):
            h0, he = _SLICE_ROWS[s]
            n = (he - h0) * W
            o = _SLICE_OFF[s]
            xpack[:, o : o + n] = xs[:, h0:he].reshape(NPART, n)
        in_maps.append(
            {
                "xpack": xpack,
                "ones_cs": ones_cs,
                "wb": wb,
                "ones_p": ones_p,
            }
        )
    return in_maps


def kernel(x, weight, bias):
    from concourse.bass_utils import run_bass_kernel_spmd

    x = np.asarray(x)
    weight = np.asarray(weight)
    bias = np.asarray(bias)
    nc = _build()
    in_maps = _host_prep(x, weight, bias)
    res = run_bass_kernel_spmd(nc, in_maps, core_ids=list(range(N_CORES)))
    out = np.concatenate(
        [
            res.results[r]["y"].astype(np.float32).reshape(BPC, COUT, H, W)
            for r in range(N_CORES)
        ],
        axis=0,
    )
    return out
